# revision 16
# baseline (speedup 1.0000x reference)
"""MultiHeadAttention (B=2, S=2048, D=1024, 16 heads, causal, torch-.view head
split) on 8 TRN2 NeuronCores.

Sharding: core c handles batch b = c//4 and heads [4g, 4g+4) with g = c%4
(head h only touches token rows [128h, 128(h+1)) of its batch, so each core
needs just 512 rows of q/k/v). Wp is row-sharded by head; each core returns a
partial (2048, 1024) output and the host sums the 4 partials per batch.

Layout notes:
- Head h's (2048, 64) matrices come from the (128 tokens x 1024 cols) block
  via s = 16*t + c, d = col%64, c = col//64. On-chip we keep head-space
  sequence order PERMUTED within each 128-tile: w = 8*c + t_lo (t = 8*j+t_lo),
  which makes all gather DMAs 32B-contiguous while preserving the causal
  block structure. The final output DMA un-permutes.
- Q/K path is float32r (fp32 storage, fast reduced-precision matmul) to keep
  softmax logits accurate; V / P / att / Wp are bf16.
"""

import numpy as np
import ml_dtypes
from contextlib import ExitStack

import concourse.bass as bass
import concourse.tile as tile
from concourse import bacc, mybir
from concourse.bass_utils import run_bass_kernel_spmd
from concourse.tile_rust import add_dep_helper
from concourse.masks import make_identity

F32 = mybir.dt.float32
F32R = mybir.dt.float32r
BF16 = mybir.dt.bfloat16
BF16_NP = ml_dtypes.bfloat16

B, S, D, NH, HD = 2, 2048, 1024, 16, 64
HPC = 4          # heads per core
ROWS = 512       # token rows per core
N_CORES = 8
EXP_FN = mybir.ActivationFunctionType.Exp


def _perm_mask_np():
    """(128,128) bf16 mask in permuted within-tile coords: mask[wk, wq] = 1
    iff s(wq) >= s(wk), with s(w) = 16*(w%8) + w//8."""
    w = np.arange(128)
    s = 16 * (w % 8) + w // 8
    m = (s[None, :] >= s[:, None]).astype(np.float32)
    return m.astype(BF16_NP)


_PROGRAM = None


def _build_program(debug_dump=False, trunc=None):
    nc = bacc.Bacc("TRN2", target_bir_lowering=False, debug=False)

    qT_d = nc.dram_tensor("qT", [D, ROWS], F32R, kind="ExternalInput").ap()
    kT_d = nc.dram_tensor("kT", [D, ROWS], F32R, kind="ExternalInput").ap()
    vT_d = nc.dram_tensor("vT", [D, ROWS], BF16, kind="ExternalInput").ap()
    Wq_d = nc.dram_tensor("Wq", [D, D], F32R, kind="ExternalInput").ap()
    Wk_d = nc.dram_tensor("Wk", [D, D], F32R, kind="ExternalInput").ap()
    Wv_d = nc.dram_tensor("Wv", [D, D], BF16, kind="ExternalInput").ap()
    Wp_d = nc.dram_tensor("Wp", [HPC * HD, D], BF16, kind="ExternalInput").ap()
    mask_d = nc.dram_tensor("mask", [128, 128], BF16, kind="ExternalInput").ap()
    out_d = nc.dram_tensor("out", [S, D], F32, kind="ExternalOutput").ap()
    dbg = {}
    if debug_dump:
        for nm, shape, dt in [
            ("dQT", [128, 8, ROWS], F32),
            ("dKT", [128, 8, ROWS], F32),
            ("dVT", [128, 8, ROWS], BF16),
            ("dQhT", [128, 2, 16, 16, 8], F32),
            ("dKhT", [128, 2, 16, 16, 8], F32),
            ("dVnat", [128, HPC, 16, HD + 1], BF16),
            ("dattT2", [128, 2, 16, 128], BF16),
            ("dPT", [128, 16, ROWS], BF16),
            ("dVpre", [128, 2, 16, 16, 8], BF16),
            ("dVstg", [128, HD], BF16),
        ]:
            dbg[nm] = nc.dram_tensor(nm, shape, dt, kind="ExternalOutput").ap()

    with tile.TileContext(nc) as tc:
        with ExitStack() as ctx:
            # ---------------- persistent tensors ----------------
            pers = ctx.enter_context(tc.tile_pool(name="pers", bufs=1))
            phaseA = ctx.enter_context(tc.tile_pool(name="phaseA", bufs=1))
            # projected X^T, block layout: [p, dblk, t] = X^T[128*dblk+p, t]
            QT_sb = phaseA.tile([128, 8, ROWS], F32R)
            KT_sb = phaseA.tile([128, 8, ROWS], F32R)
            VT_sb = phaseA.tile([128, 8, ROWS], BF16)
            # head-gathered, pair-packed: [64*(h%2)+d, h//2, j, c, t_lo]
            QhT = pers.tile([128, 2, 16, 16, 8], F32R)
            KhT = pers.tile([128, 2, 16, 16, 8], F32R)
            V_pre = phaseA.tile([128, 2, 16, 16, 8], BF16)
            # V natural per head + ones column: [w, hl, j, 0:65]
            V_nat = pers.tile([128, HPC, 16, HD + 1], BF16)
            # att^T pair-packed for Wp: [64*(h%2)+d, h//2, qt, wq]
            attT2 = pers.tile([128, 2, 16, 128], BF16)
            Wp_sb = pers.tile([128, 2, D], BF16)
            mask_t = pers.tile([128, 128], BF16)
            ident = pers.tile([128, 128], BF16)
            make_identity(nc, ident)

            nc.sync.dma_start(out=mask_t, in_=mask_d)
            nc.sync.dma_start(
                out=Wp_sb, in_=Wp_d.rearrange("(a p) e -> p a e", p=128)
            )
            nc.gpsimd.memset(V_nat[:, :, :, HD : HD + 1], 1.0)

            # ---------------- projections ----------------
            ps512 = ctx.enter_context(
                tc.tile_pool(name="ps512", bufs=4, space="PSUM")
            )
            with tc.tile_pool(name="xin", bufs=1) as xin_pool, tc.tile_pool(
                name="wcol", bufs=3
            ) as w_pool, tc.tile_pool(name="evq", bufs=1) as _:
                proj = [
                    (qT_d, Wq_d, QT_sb, F32R),
                    (kT_d, Wk_d, KT_sb, F32R),
                    (vT_d, Wv_d, VT_sb, BF16),
                ]
                for xd, wd, xt_out, wdt in proj:
                    x_in = xin_pool.tile([128, 8, ROWS], wdt, tag="x_in")
                    nc.sync.dma_start(
                        out=x_in, in_=xd.rearrange("(a p) t -> p a t", p=128)
                    )
                    for dblk in range(8):
                        wcol = w_pool.tile([128, 8, 128], wdt, tag="wcol")
                        nc.sync.dma_start(
                            out=wcol,
                            in_=wd[:, 128 * dblk : 128 * (dblk + 1)].rearrange(
                                "(a p) d -> p a d", p=128
                            ),
                        )
                        psum = ps512.tile([128, ROWS], F32, tag="ps512")
                        for mt in range(8):
                            nc.tensor.matmul(
                                psum,
                                lhsT=wcol[:, mt, :],
                                rhs=x_in[:, mt, :],
                                start=(mt == 0),
                                stop=(mt == 7),
                            )
                        dst = xt_out[:, dblk, :]
                        if dst.dtype == F32R:
                            dst = dst.bitcast(F32)
                        nc.any.tensor_copy(dst, psum)

            # ---------------- head gathers (SBUF->SBUF DMA) ----------------
            # The xbar DMA-transpose corrupts when plain DMAs run on other
            # queues concurrently (shared xbar mode), so run every transpose
            # strictly AFTER the last plain gather DMA via explicit deps.
            vs_pool = ctx.enter_context(tc.tile_pool(name="vstg", bufs=20))
            last_gather = None
            for hl in range(HPC):
                hp, ho = hl // 2, (hl % 2) * 64
                for c in range(16):
                    p0 = 64 * (c % 2)
                    src_q = QT_sb[
                        p0 : p0 + 64, c // 2, 128 * hl : 128 * (hl + 1)
                    ].rearrange("p (j w) -> p j w", w=8)
                    nc.sync.dma_start(out=QhT[ho : ho + 64, hp, :, c, :], in_=src_q)
                    src_k = KT_sb[
                        p0 : p0 + 64, c // 2, 128 * hl : 128 * (hl + 1)
                    ].rearrange("p (j w) -> p j w", w=8)
                    nc.sync.dma_start(out=KhT[ho : ho + 64, hp, :, c, :], in_=src_k)
                    src_v = VT_sb[
                        p0 : p0 + 64, c // 2, 128 * hl : 128 * (hl + 1)
                    ].rearrange("p (j w) -> p j w", w=8)
                    last_gather = nc.sync.dma_start(
                        out=V_pre[ho : ho + 64, hp, :, c, :], in_=src_v
                    )
            for hl in range(HPC):
                hp, ho = hl // 2, (hl % 2) * 64
                # V: transpose (64d x 128w) -> (128w x 64d) per k-tile.
                # dma_start_transpose needs a contiguous dest, so stage then
                # copy into the 65-strided V_nat.
                for j in range(16):
                    vstg = vs_pool.tile([128, HD], BF16, tag="vstg")
                    tr = nc.sync.dma_start_transpose(
                        out=vstg,
                        in_=V_pre[ho : ho + 64, hp, j, :, :],
                    )
                    add_dep_helper(tr.ins, last_gather.ins,
                                   reason="xbar transpose after plain DMAs")
                    nc.vector.tensor_copy(V_nat[:, hl, j, 0:HD], vstg)
                    if debug_dump and hl == 0 and j == 0:
                        nc.sync.dma_start(out=dbg["dVstg"], in_=vstg)

            if debug_dump:
                nc.sync.dma_start(out=dbg["dQT"], in_=QT_sb.bitcast(F32))
                nc.sync.dma_start(out=dbg["dKT"], in_=KT_sb.bitcast(F32))
                nc.sync.dma_start(out=dbg["dVT"], in_=VT_sb)
                nc.sync.dma_start(out=dbg["dQhT"], in_=QhT.bitcast(F32))
                nc.sync.dma_start(out=dbg["dKhT"], in_=KhT.bitcast(F32))
                nc.sync.dma_start(out=dbg["dVnat"], in_=V_nat)
                nc.sync.dma_start(out=dbg["dVpre"], in_=V_pre)
            skip_attn = trunc == "gather"

            # ---------------- attention + output projection ----------------
            if not skip_attn:
                att_ps = ctx.enter_context(
                    tc.tile_pool(name="attps", bufs=4, space="PSUM")
                )
                PT_arr = []
                for i in range(2):
                    pt_half_a = pers.tile([128, 16, ROWS], BF16, tag=f"pt{i}a")
                    pt_half_b = pers.tile([128, 16, ROWS], BF16, tag=f"pt{i}b")
                    PT_arr.append([pt_half_a, pt_half_b])
                sm_pool = ctx.enter_context(tc.tile_pool(name="small", bufs=8))
                an_pool = ctx.enter_context(tc.tile_pool(name="attn", bufs=4))
                out_pool = ctx.enter_context(tc.tile_pool(name="outt", bufs=4))

            def st_exp(qc, hpair, kt, phase):
                """S^T matmul + exp (+ diagonal mask) for both heads of the
                pair into PT_arr[phase % 2] slots (kt, half)."""
                qoff = max(0, 128 * kt - 512 * qc)
                pts = []
                for half in range(2):
                    ho = 64 * half
                    psS = ps512.tile([128, ROWS], F32, tag="ps512")
                    nc.tensor.matmul(
                        psS[:, qoff:512],
                        lhsT=KhT[ho : ho + 64, hpair, kt, :, :],
                        rhs=QhT[
                            ho : ho + 64,
                            hpair,
                            4 * qc + qoff // 128 : 4 * (qc + 1),
                            :,
                            :,
                        ],
                        start=True,
                        stop=True,
                    )
                    PT = PT_arr[phase % 2][half][:, kt, :]
                    nc.scalar.activation(PT[:, qoff:512], psS[:, qoff:512], EXP_FN)
                    if kt >= 4 * qc:  # diagonal tile
                        nc.vector.tensor_mul(
                            PT[:, qoff : qoff + 128],
                            PT[:, qoff : qoff + 128],
                            mask_t,
                        )
                    pts.append(PT)
                return pts

            n_qc = 0 if skip_attn else (1 if trunc == "attn1" else 4)
            for qc in range(n_qc):
                for hpair in range(2):
                    phase = 2 * qc + hpair
                    pts = {}
                    for kt in range(4 * qc + 1):
                        pts[kt] = st_exp(qc, hpair, kt, phase)
                    for s in range(4):
                        if s > 0:
                            pts[4 * qc + s] = st_exp(qc, hpair, 4 * qc + s, phase)
                        attn2 = an_pool.tile([128, 128], BF16, tag="attn2")
                        for half in range(2):
                            hl = 2 * hpair + half
                            acc = att_ps.tile([128, HD + 1], F32, tag="acc")
                            for kt in range(4 * qc + s + 1):
                                nc.tensor.matmul(
                                    acc,
                                    lhsT=pts[kt][half][:, 128 * s : 128 * (s + 1)],
                                    rhs=V_nat[:, hl, kt, :],
                                    start=(kt == 0),
                                    stop=(kt == 4 * qc + s),
                                )
                            recip = sm_pool.tile([128, 1], F32, tag="recip")
                            nc.vector.reciprocal(recip, acc[:, HD : HD + 1])
                            nc.vector.tensor_scalar_mul(
                                attn2[:, 64 * half : 64 * (half + 1)],
                                acc[:, 0:HD],
                                recip,
                            )
                        ps_t = ps512.tile([128, 128], BF16, tag="ps512")
                        nc.tensor.transpose(ps_t, attn2, ident)
                        nc.any.tensor_copy(attT2[:, hpair, 4 * qc + s, :], ps_t)
                # Wp for this chunk's 4 q-tiles
                for s in range(4):
                    qt = 4 * qc + s
                    for ec in range(2):
                        ps_o = ps512.tile([128, ROWS], F32, tag="ps512")
                        for pair in range(2):
                            nc.tensor.matmul(
                                ps_o,
                                lhsT=attT2[:, pair, qt, :],
                                rhs=Wp_sb[:, pair, 512 * ec : 512 * (ec + 1)],
                                start=(pair == 0),
                                stop=(pair == 1),
                            )
                        out_t = out_pool.tile([128, ROWS], F32, tag="out_t")
                        nc.any.tensor_copy(out_t, ps_o)
                        # un-permute rows: partition w=8c+tl -> row 16*tl+c.
                        # DRAM-side AP traversal (c outer, tl inner) matches
                        # the SBUF partition order w = 8c+tl.
                        dst = out_d[
                            128 * qt : 128 * (qt + 1), 512 * ec : 512 * (ec + 1)
                        ].rearrange("(tl c) e -> c tl e", tl=8)
                        nc.sync.dma_start(out=dst, in_=out_t)
            if debug_dump and not skip_attn:
                nc.sync.dma_start(out=dbg["dattT2"], in_=attT2)
                nc.sync.dma_start(out=dbg["dPT"], in_=PT_arr[2 * (n_qc - 1) + 1][0] if False else PT_arr[1][0])

    nc.compile()
    return nc


def get_program(debug_dump=False, trunc=None):
    global _PROGRAM
    if _PROGRAM is None:
        _PROGRAM = _build_program(debug_dump, trunc)
    return _PROGRAM


def make_in_maps(q, k, v, Wq, Wk, Wv, Wp):
    mask = _perm_mask_np()
    Wq = np.asarray(Wq, np.float32)
    Wk = np.asarray(Wk, np.float32)
    Wv_b = np.asarray(Wv, np.float32).astype(BF16_NP)
    Wp_f = np.asarray(Wp, np.float32)
    in_maps = []
    for core in range(N_CORES):
        b, g = divmod(core, 4)
        rows = slice(ROWS * g, ROWS * (g + 1))
        in_maps.append(
            {
                "qT": np.ascontiguousarray(np.asarray(q[b], np.float32)[rows].T),
                "kT": np.ascontiguousarray(np.asarray(k[b], np.float32)[rows].T),
                "vT": np.ascontiguousarray(
                    np.asarray(v[b], np.float32)[rows].T
                ).astype(BF16_NP),
                "Wq": Wq,
                "Wk": Wk,
                "Wv": Wv_b,
                "Wp": np.ascontiguousarray(
                    Wp_f[HPC * HD * g : HPC * HD * (g + 1)]
                ).astype(BF16_NP),
                "mask": mask,
            }
        )
    return in_maps


def kernel(q, k, v, Wq, Wk, Wv, Wp, _trace=False, _trace_kwargs=None):
    nc = get_program()
    in_maps = make_in_maps(q, k, v, Wq, Wk, Wv, Wp)
    res = run_bass_kernel_spmd(
        nc,
        in_maps,
        core_ids=list(range(N_CORES)),
        trace=_trace,
        **(_trace_kwargs or {}),
    )
    outs = [res.results[c]["out"] for c in range(N_CORES)]
    full = np.stack(
        [
            outs[0] + outs[1] + outs[2] + outs[3],
            outs[4] + outs[5] + outs[6] + outs[7],
        ]
    ).astype(np.float32)
    if _trace:
        kernel._last_result = res
    return full


# revision 23
# speedup vs baseline: 2.2168x; 2.2168x over previous
"""MultiHeadAttention (B=2, S=2048, D=1024, 16 heads, causal, torch-.view head
split) on 8 TRN2 NeuronCores.

Sharding: core c handles batch b = c//4 and heads [4g, 4g+4) with g = c%4
(head h only touches token rows [128h, 128(h+1)) of its batch, so each core
needs just 512 rows of q/k/v). Wp is row-sharded by head; each core returns a
partial (2048, 1024) output and the host sums the 4 partials per batch.

Layout notes:
- Head h's (2048, 64) matrices come from the (128 tokens x 1024 cols) block
  via s = 16*t + c, d = col%64, c = col//64. On-chip we keep head-space
  sequence order PERMUTED within each 128-tile: w = 8*c + t_lo (t = 8*j+t_lo),
  which makes all gather DMAs 32B-contiguous while preserving the causal
  block structure. The final output DMA un-permutes.
- Everything is bf16 except PSUM accumulation, softmax denominators and the
  final output (f32).
"""

import numpy as np
import ml_dtypes
from contextlib import ExitStack

import concourse.bass as bass
import concourse.tile as tile
from concourse import bacc, mybir
from concourse.bass_utils import run_bass_kernel_spmd
from concourse.masks import make_identity

F32 = mybir.dt.float32
F16 = mybir.dt.float16
F16_NP = np.float16
BF16 = mybir.dt.bfloat16
BF16_NP = ml_dtypes.bfloat16

B, S, D, NH, HD = 2, 2048, 1024, 16, 64
HPC = 4          # heads per core
ROWS = 512       # token rows per core
N_CORES = 8
EXP_FN = mybir.ActivationFunctionType.Exp


def _perm_mask_np():
    """(128,128) bf16 mask in permuted within-tile coords: mask[wk, wq] = 1
    iff s(wq) >= s(wk), with s(w) = 16*(w%8) + w//8."""
    w = np.arange(128)
    s = 16 * (w % 8) + w // 8
    m = (s[None, :] >= s[:, None]).astype(np.float32)
    return m.astype(BF16_NP)


_PROGRAM = None


def _build_program(debug_dump=False, trunc=None):
    nc = bacc.Bacc("TRN2", target_bir_lowering=False, debug=False)

    qT_d = nc.dram_tensor("qT", [D, ROWS], F16, kind="ExternalInput").ap()
    kT_d = nc.dram_tensor("kT", [D, ROWS], F16, kind="ExternalInput").ap()
    vT_d = nc.dram_tensor("vT", [D, ROWS], BF16, kind="ExternalInput").ap()
    Wq_d = nc.dram_tensor("Wq", [D, D], F16, kind="ExternalInput").ap()
    Wk_d = nc.dram_tensor("Wk", [D, D], F16, kind="ExternalInput").ap()
    Wv_d = nc.dram_tensor("Wv", [D, D], BF16, kind="ExternalInput").ap()
    Wp_d = nc.dram_tensor("Wp", [HPC * HD, D], BF16, kind="ExternalInput").ap()
    mask_d = nc.dram_tensor("mask", [128, 128], BF16, kind="ExternalInput").ap()
    out_d = nc.dram_tensor("out", [S, D], F32, kind="ExternalOutput").ap()
    dbg = {}
    if debug_dump:
        for nm, shape, dt in [
            ("dQT", [128, 8, ROWS], F16),
            ("dKT", [128, 8, ROWS], F16),
            ("dVT", [128, 8, ROWS], BF16),
            ("dQhT", [128, 2, 16, 16, 8], F16),
            ("dKhT", [128, 2, 16, 16, 8], F16),
            ("dVnat", [128, HPC, 16, HD + 1], BF16),
            ("dattT2", [128, 2, 16, 128], BF16),
            ("dPT", [128, 16, ROWS], BF16),
        ]:
            dbg[nm] = nc.dram_tensor(nm, shape, dt, kind="ExternalOutput").ap()

    with tile.TileContext(nc) as tc:
        with ExitStack() as ctx:
            # ---------------- persistent tensors ----------------
            pers = ctx.enter_context(tc.tile_pool(name="pers", bufs=1))
            phaseA = ctx.enter_context(tc.tile_pool(name="phaseA", bufs=1))
            # projected X^T, block layout: [p, dblk, t] = X^T[128*dblk+p, t]
            QT_sb = phaseA.tile([128, 8, ROWS], F16)
            KT_sb = phaseA.tile([128, 8, ROWS], F16)
            VT_sb = phaseA.tile([128, 8, ROWS], BF16)
            # head-gathered, pair-packed: [64*(h%2)+d, h//2, j, c, t_lo]
            QhT = pers.tile([128, 2, 16, 16, 8], F16)
            KhT = pers.tile([128, 2, 16, 16, 8], F16)
            V_pre = phaseA.tile([128, 2, 16, 16, 8], BF16)
            # partition-half-swapped copies of the projections
            QT_sw = phaseA.tile([128, 8, ROWS], F16)
            KT_sw = phaseA.tile([128, 8, ROWS], F16)
            VT_sw = phaseA.tile([128, 8, ROWS], BF16)
            # V natural per head + ones column: [w, hl, j, 0:65]
            V_nat = pers.tile([128, HPC, 16, HD + 1], BF16)
            # att^T pair-packed for Wp: [64*(h%2)+d, h//2, qt, wq]
            attT2 = pers.tile([128, 2, 16, 128], BF16)
            Wp_sb = pers.tile([128, 2, D], BF16)
            mask_t = pers.tile([128, 128], BF16)
            ident = pers.tile([128, 128], BF16)
            make_identity(nc, ident)

            nc.sync.dma_start(out=mask_t, in_=mask_d)
            nc.sync.dma_start(
                out=Wp_sb, in_=Wp_d.rearrange("(a p) e -> p a e", p=128)
            )
            nc.gpsimd.memset(V_nat[:, :, :, HD : HD + 1], 1.0)

            # ---------------- projections (order: v, k, q) ----------------
            ps512 = ctx.enter_context(
                tc.tile_pool(name="ps512", bufs=4, space="PSUM")
            )
            pst = ctx.enter_context(tc.tile_pool(name="pst", bufs=2, space="PSUM"))

            def gather_batch(dst, src_sb, src_sw):
                """Head gather dst[64par+d, hp, j, c, tl] =
                src[64(c%2)+d, c//2, 128hl + 8j + tl] as partition-aligned
                ENGINE copies (multi-dim free APs), reading the half-swapped
                copy when par != c%2. 8 copies per tensor, split DVE/GpSimd."""
                i = 0
                for hl in range(HPC):
                    par, hp = hl % 2, hl // 2
                    po = 64 * par
                    for c0 in range(2):
                        srct = src_sb if par == c0 else src_sw
                        inv = srct[
                            po : po + 64, :, 128 * hl : 128 * (hl + 1)
                        ].rearrange("d a (j w) -> d j a w", w=8)
                        outv = dst[po : po + 64, hp].rearrange(
                            "d j (cc c2) w -> d j cc c2 w", c2=2
                        )[:, :, :, c0, :]
                        eng = nc.vector if i % 2 == 0 else nc.gpsimd
                        eng.tensor_copy(outv, inv)
                        i += 1

            with tc.tile_pool(name="xin", bufs=1) as xin_pool, tc.tile_pool(
                name="wcol", bufs=3
            ) as w_pool:
                proj = [
                    (qT_d, Wq_d, QT_sb, QT_sw, F16),
                    (kT_d, Wk_d, KT_sb, KT_sw, F16),
                    (vT_d, Wv_d, VT_sb, VT_sw, BF16),
                ]
                for xd, wd, xt_out, xt_sw, xdt in proj:
                    x_in = xin_pool.tile([128, 8, ROWS], xdt, tag="x_in")
                    nc.sync.dma_start(
                        out=x_in, in_=xd.rearrange("(a p) t -> p a t", p=128)
                    )
                    for dblk in range(8):
                        wcol = w_pool.tile([128, 8, 128], xdt, tag="wcol")
                        nc.sync.dma_start(
                            out=wcol,
                            in_=wd[:, 128 * dblk : 128 * (dblk + 1)].rearrange(
                                "(a p) d -> p a d", p=128
                            ),
                        )
                        psum = ps512.tile([128, ROWS], F32, tag="ps512")
                        for mt in range(8):
                            nc.tensor.matmul(
                                psum,
                                lhsT=wcol[:, mt, :],
                                rhs=x_in[:, mt, :],
                                start=(mt == 0),
                                stop=(mt == 7),
                            )
                        if xt_out is VT_sb:
                            nc.vector.tensor_copy(xt_out[:, dblk, :], psum)
                        else:
                            nc.scalar.copy(xt_out[:, dblk, :], psum)
                    # half-swap copy, then gathers (engine copies)
                    nc.sync.dma_start(out=xt_sw[0:64], in_=xt_out[64:128])
                    nc.sync.dma_start(out=xt_sw[64:128], in_=xt_out[0:64])
                    if xt_out is VT_sb:
                        gather_batch(V_pre, VT_sb, VT_sw)
                        for hl in range(HPC):
                            hp, ho = hl // 2, (hl % 2) * 64
                            for j in range(16):
                                ps_v = pst.tile([128, HD], BF16, tag="pst")
                                nc.tensor.transpose(
                                    ps_v,
                                    V_pre[ho : ho + 64, hp, j, :, :],
                                    ident[ho : ho + 64, ho : ho + 64],
                                )
                                nc.vector.tensor_copy(
                                    V_nat[:, hl, j, 0:HD], ps_v
                                )
                    elif xt_out is KT_sb:
                        gather_batch(KhT, KT_sb, KT_sw)
                    else:
                        gather_batch(QhT, QT_sb, QT_sw)

            if debug_dump:
                nc.sync.dma_start(out=dbg["dQT"], in_=QT_sb)
                nc.sync.dma_start(out=dbg["dKT"], in_=KT_sb)
                nc.sync.dma_start(out=dbg["dVT"], in_=VT_sb)
                nc.sync.dma_start(out=dbg["dQhT"], in_=QhT)
                nc.sync.dma_start(out=dbg["dKhT"], in_=KhT)
                nc.sync.dma_start(out=dbg["dVnat"], in_=V_nat)
            skip_attn = trunc == "gather"

            # ---------------- attention + output projection ----------------
            if not skip_attn:
                att_ps = ctx.enter_context(
                    tc.tile_pool(name="attps", bufs=2, space="PSUM")
                )
                PT_arr = []
                for i in range(2):
                    pt_half_a = pers.tile([128, 16, ROWS], BF16, tag=f"pt{i}a")
                    pt_half_b = pers.tile([128, 16, ROWS], BF16, tag=f"pt{i}b")
                    PT_arr.append([pt_half_a, pt_half_b])
                sm_pool = ctx.enter_context(tc.tile_pool(name="small", bufs=8))
                an_pool = ctx.enter_context(tc.tile_pool(name="attn", bufs=4))
                out_pool = ctx.enter_context(tc.tile_pool(name="outt", bufs=4))

            def st_exp(qc, hpair, kt, phase):
                """S^T matmul + exp (+ diagonal mask) for both heads of the
                pair into PT_arr[phase % 2] slots (kt, half)."""
                qoff = max(0, 128 * kt - 512 * qc)
                pts = []
                for half in range(2):
                    ho = 64 * half
                    psS = ps512.tile([128, ROWS], F32, tag="ps512")
                    nc.tensor.matmul(
                        psS[:, qoff:512],
                        lhsT=KhT[ho : ho + 64, hpair, kt, :, :],
                        rhs=QhT[
                            ho : ho + 64, hpair,
                            4 * qc + qoff // 128 : 4 * (qc + 1), :, :,
                        ],
                        start=True,
                        stop=True,
                    )
                    PT = PT_arr[phase % 2][half][:, kt, :]
                    nc.scalar.activation(PT[:, qoff:512], psS[:, qoff:512], EXP_FN)
                    if kt >= 4 * qc:  # diagonal tile
                        nc.vector.tensor_mul(
                            PT[:, qoff : qoff + 128],
                            PT[:, qoff : qoff + 128],
                            mask_t,
                        )
                    pts.append(PT)
                return pts

            n_qc = 0 if skip_attn else (1 if trunc == "attn1" else 4)
            for qc in range(n_qc):
                for hpair in range(2):
                    phase = 2 * qc + hpair
                    pts = {}
                    for kt in range(4 * qc + 1):
                        pts[kt] = st_exp(qc, hpair, kt, phase)
                    for s in range(4):
                        if s > 0:
                            pts[4 * qc + s] = st_exp(qc, hpair, 4 * qc + s, phase)
                        attn2 = an_pool.tile([128, 128], BF16, tag="attn2")
                        for half in range(2):
                            hl = 2 * hpair + half
                            acc = att_ps.tile([128, HD + 1], F32, tag="acc")
                            for kt in range(4 * qc + s + 1):
                                nc.tensor.matmul(
                                    acc,
                                    lhsT=pts[kt][half][:, 128 * s : 128 * (s + 1)],
                                    rhs=V_nat[:, hl, kt, :],
                                    start=(kt == 0),
                                    stop=(kt == 4 * qc + s),
                                )
                            recip = sm_pool.tile([128, 1], F32, tag="recip")
                            nc.vector.reciprocal(recip, acc[:, HD : HD + 1])
                            nc.vector.tensor_scalar_mul(
                                attn2[:, 64 * half : 64 * (half + 1)],
                                acc[:, 0:HD],
                                recip,
                            )
                        ps_t = pst.tile([128, 128], BF16, tag="pst")
                        nc.tensor.transpose(ps_t, attn2, ident)
                        nc.vector.tensor_copy(attT2[:, hpair, 4 * qc + s, :], ps_t)
                # Wp for this chunk's 4 q-tiles
                for s in range(4):
                    qt = 4 * qc + s
                    for ec in range(2):
                        ps_o = ps512.tile([128, ROWS], F32, tag="ps512")
                        for pair in range(2):
                            nc.tensor.matmul(
                                ps_o,
                                lhsT=attT2[:, pair, qt, :],
                                rhs=Wp_sb[:, pair, 512 * ec : 512 * (ec + 1)],
                                start=(pair == 0),
                                stop=(pair == 1),
                            )
                        out_t = out_pool.tile([128, ROWS], F32, tag="out_t")
                        nc.vector.tensor_copy(out_t, ps_o)
                        # un-permute rows: partition w=8c+tl -> row 16*tl+c.
                        # DRAM-side AP traversal (c outer, tl inner) matches
                        # the SBUF partition order w = 8c+tl.
                        dst = out_d[
                            128 * qt : 128 * (qt + 1), 512 * ec : 512 * (ec + 1)
                        ].rearrange("(tl c) e -> c tl e", tl=8)
                        nc.sync.dma_start(out=dst, in_=out_t)
            if debug_dump and not skip_attn:
                nc.sync.dma_start(out=dbg["dattT2"], in_=attT2)
                nc.sync.dma_start(out=dbg["dPT"], in_=PT_arr[1][0])

    nc.compile()
    return nc


def get_program(debug_dump=False, trunc=None):
    global _PROGRAM
    if _PROGRAM is None:
        _PROGRAM = _build_program(debug_dump, trunc)
    return _PROGRAM


def make_in_maps(q, k, v, Wq, Wk, Wv, Wp):
    mask = _perm_mask_np()
    Wq_b = np.asarray(Wq, np.float32).astype(F16_NP)
    Wk_b = np.asarray(Wk, np.float32).astype(F16_NP)
    Wv_b = np.asarray(Wv, np.float32).astype(BF16_NP)
    Wp_f = np.asarray(Wp, np.float32)
    in_maps = []
    for core in range(N_CORES):
        b, g = divmod(core, 4)
        rows = slice(ROWS * g, ROWS * (g + 1))
        in_maps.append(
            {
                "qT": np.ascontiguousarray(
                    np.asarray(q[b], np.float32)[rows].T
                ).astype(F16_NP),
                "kT": np.ascontiguousarray(
                    np.asarray(k[b], np.float32)[rows].T
                ).astype(F16_NP),
                "vT": np.ascontiguousarray(
                    np.asarray(v[b], np.float32)[rows].T
                ).astype(BF16_NP),
                "Wq": Wq_b,
                "Wk": Wk_b,
                "Wv": Wv_b,
                "Wp": np.ascontiguousarray(
                    Wp_f[HPC * HD * g : HPC * HD * (g + 1)]
                ).astype(BF16_NP),
                "mask": mask,
            }
        )
    return in_maps


def kernel(q, k, v, Wq, Wk, Wv, Wp, _trace=False, _trace_kwargs=None):
    nc = get_program()
    in_maps = make_in_maps(q, k, v, Wq, Wk, Wv, Wp)
    res = run_bass_kernel_spmd(
        nc,
        in_maps,
        core_ids=list(range(N_CORES)),
        trace=_trace,
        **(_trace_kwargs or {}),
    )
    outs = [res.results[c]["out"] for c in range(N_CORES)]
    full = np.stack(
        [
            outs[0] + outs[1] + outs[2] + outs[3],
            outs[4] + outs[5] + outs[6] + outs[7],
        ]
    ).astype(np.float32)
    if _trace:
        kernel._last_result = res
    return full
